# revision 2
# baseline (speedup 1.0000x reference)
"""CapsuleLayer dynamic-routing kernel, v5: mixed n/i-sharding, 8 cores.

Sharding: core c owns n-block c (128 n's, all 8 i's) END-TO-END: its
s-chain rows (i, n) are 8 tiles, and its routing state (a, b, softmax
c, Wc) is fully LOCAL -- no collective in the b-update loop. The 9th
n-block (n in [1024,1152)) is i-sharded v4-style: each core contracts
(n8, i=c) as one extra full-width tile per chain; its a-partial needs
a tiny AllReduce [128,10] which hides behind the main-block work.

Per iteration: 9-tile fp32 s-chain per batch-half (AR(s) per half
overlaps the other half / squash / P), squash replicated on [128,320],
P per (i)-tile (2 matmuls each), DVE product + (i,o)-reduce, local b /
softmax / Wc build. it0 uses host-precomputed 0.1*W (softmax of
zeros), so the next rep's it0 chain overlaps this rep's it2 tail.
"""

import os as _os
import numpy as np

B, N, C, O, I = 256, 1152, 10, 16, 8
NCORES = 8
KO = C * O            # 160
NH = 2                # batch halves of 128
NI = 8                # i-tiles of the main block
NG_A = 3              # a-reduce bank groups
ITERS = 3

# engine knobs: "v"=DVE, "s"=ScalarE/ACT, "g"=Pool
PROD_ENG = _os.environ.get("K_PROD", "v")
RED_ENG = _os.environ.get("K_RED", "v")
WC_ENG = _os.environ.get("K_WC", "vg")   # [first-half, second-half] of i
WC8_ENG = _os.environ.get("K_WC8", "g")

_BUILT = {}


def _build_program(num_devices=NCORES, collective=True, repeat=1):
    import concourse.bass as bass
    import concourse.mybir as mybir
    import concourse.tile as tile
    import concourse.bacc as bacc

    f32 = mybir.dt.float32
    AX = mybir.AxisListType
    ALU = mybir.AluOpType
    ACT = mybir.ActivationFunctionType

    nc = bacc.Bacc("TRN2", target_bir_lowering=False, debug=False,
                   num_devices=num_devices)

    if repeat > 1:
        nc.dram_tensor("rep_tag", [1, repeat], f32, kind="ExternalInput")
    # main block stationary x for s: [p=n, (i, h, bcol)]
    xm_d = nc.dram_tensor("x_m", [128, NI * NH * 128], f32,
                          kind="ExternalInput")
    # block-8 stationary x for s (i=c): [p=n8, (h, bcol)]
    x8s_d = nc.dram_tensor("x8s", [128, NH * 128], f32,
                           kind="ExternalInput")
    # main block stationary x for P: [p=b, (i, h, ncol)]
    xpm_d = nc.dram_tensor("x_pm", [128, NI * NH * 128], f32,
                           kind="ExternalInput")
    # block-8 stationary x for P (all i): [p=b, (i, h, n8col)]
    x8p_d = nc.dram_tensor("x8p", [128, NI * NH * 128], f32,
                           kind="ExternalInput")
    # W main [p=n, (i, k, o)]; W8 s-slice (i=c) and full (all i)
    wm_d = nc.dram_tensor("w_m", [128, NI * KO], f32, kind="ExternalInput")
    w8_d = nc.dram_tensor("w_8", [128, KO], f32, kind="ExternalInput")
    w8f_d = nc.dram_tensor("w_8f", [128, NI * KO], f32,
                           kind="ExternalInput")
    w0m_d = nc.dram_tensor("w0_m", [128, NI * KO], f32, kind="ExternalInput")
    w08_d = nc.dram_tensor("w0_8", [128, KO], f32, kind="ExternalInput")
    v_d = nc.dram_tensor("v_out", [128, NH * KO], f32, kind="ExternalOutput")

    def eng(nm):
        return {"v": nc.vector, "s": nc.scalar, "g": nc.gpsimd}[nm]

    with tile.TileContext(nc) as tc:
        with (
            tc.tile_pool(name="main", bufs=1) as pool,
            tc.tile_pool(name="pp", bufs=1, space="PSUM") as pp,
            tc.tile_pool(name="ps", bufs=2, space="PSUM") as ps,
            tc.tile_pool(name="dram", bufs=2, space="DRAM") as dram,
        ):
            x_m = pool.tile([128, NI * NH * 128], f32)
            x8s = pool.tile([128, NH * 128], f32)
            x_pm = pool.tile([128, NI * NH * 128], f32)
            x8p = pool.tile([128, NI * NH * 128], f32)
            w_m = pool.tile([128, NI * KO], f32)
            w_8 = pool.tile([128, KO], f32)
            w_8f = pool.tile([128, NI * KO], f32)
            w0m = pool.tile([128, NI * KO], f32)
            w08 = pool.tile([128, KO], f32)
            wc_m = pool.tile([128, NI * KO], f32)
            wc8 = pool.tile([128, KO], f32)
            b_m = pool.tile([128, C], f32)
            b_8 = pool.tile([128, C], f32)

            for t, d in [(x_m, xm_d), (x8s, x8s_d), (x_pm, xpm_d),
                         (x8p, x8p_d), (w_m, wm_d), (w_8, w8_d),
                         (w_8f, w8f_d), (w0m, w0m_d), (w08, w08_d)]:
                nc.sync.dma_start(t[:, :], d[:, :])

            wmv = w_m.rearrange("p (i k o) -> p i k o", i=NI, k=C)
            wcmv = wc_m.rearrange("p (i k o) -> p i k o", i=NI, k=C)

            # squash scratch (both halves side by side)
            sq_t = pool.tile([128, NH * KO], f32, tag="sq_sq")
            sg_t = pool.tile([128, NH * KO], f32, tag="sq_sg")
            rc_t = pool.tile([128, NH * KO], f32, tag="sq_rc")
            m_t = pool.tile([128, NH * KO], f32, tag="sq_m")

            state = {}

            def emit_front(it):
                """s-chain + AR + squash -> v_sb for iteration `it`."""
                wm_src = w0m if it == 0 else wc_m
                w8_src = w08 if it == 0 else wc8
                s_ps = ps.tile([128, NH * KO], f32, tag="s_ps", bufs=2)
                s_sb = pool.tile([128, NH * KO], f32, tag="s_sb", bufs=2)
                s_pre = pool.tile([128, NH * KO], f32, tag="s_pre", bufs=2)
                for h in range(NH):
                    hs = slice(h * KO, (h + 1) * KO)
                    for i in range(NI):
                        nc.tensor.matmul(
                            s_ps[:, hs],
                            x_m[:, (i * NH + h) * 128:(i * NH + h + 1) * 128],
                            wm_src[:, i * KO:(i + 1) * KO],
                            start=(i == 0), stop=False)
                    nc.tensor.matmul(
                        s_ps[:, hs], x8s[:, h * 128:(h + 1) * 128],
                        w8_src[:, :], start=False, stop=True)
                    nc.scalar.activation(s_pre[:, hs], s_ps[:, hs], ACT.Copy)
                s_ci = dram.tile([128, NH * KO], f32, tag="s_ci")
                nc.sync.dma_start(s_ci[:, :], s_pre[:, :])
                if collective:
                    s_co = dram.tile([128, NH * KO], f32, tag="s_co",
                                     addr_space="Shared")
                    nc.gpsimd.collective_compute(
                        "AllReduce", ALU.add,
                        replica_groups=[list(range(num_devices))],
                        ins=[s_ci.opt()], outs=[s_co.opt()])
                    nc.sync.dma_start(s_sb[:, :], s_co[:, :])
                else:
                    nc.sync.dma_start(s_sb[:, :], s_ci[:, :])
                v_sb = pool.tile([128, NH * KO], f32, tag="v_sb", bufs=2)
                nc.scalar.activation(sq_t[:, :], s_sb[:, :], ACT.Square)
                nc.scalar.activation(sg_t[:, :], s_sb[:, :], ACT.Sign)
                nc.vector.tensor_scalar_add(rc_t[:, :], sq_t[:, :], 1.0)
                nc.vector.reciprocal_approx_fast(rc_t[:, :], rc_t[:, :])
                nc.vector.tensor_mul(m_t[:, :], sq_t[:, :], rc_t[:, :])
                nc.vector.tensor_mul(v_sb[:, :], m_t[:, :], sg_t[:, :])
                return v_sb

            def emit_a_phase(it, v_sb):
                """P matmuls + product/reduce + b update + softmax + Wc."""
                p_ps = pp.tile([128, 1536], f32, tag="p_ps", bufs=1)
                p8_ps = pp.tile([128, 1536], f32, tag="p8_ps", bufs=1)
                for i in range(NI):
                    off = (i // 3) * 512 + (i % 3) * KO
                    for h in range(NH):
                        hh = slice(h * KO, (h + 1) * KO)
                        nc.tensor.matmul(
                            p_ps[:, off:off + KO],
                            x_pm[:, (i * NH + h) * 128:(i * NH + h + 1) * 128],
                            v_sb[:, hh], start=(h == 0), stop=(h == 1))
                        nc.tensor.matmul(
                            p8_ps[:, off:off + KO],
                            x8p[:, (i * NH + h) * 128:(i * NH + h + 1) * 128],
                            v_sb[:, hh], start=(h == 0), stop=(h == 1))

                awc = pool.tile([128, NI * KO], f32, tag="awc")
                a_g = pool.tile([128, NG_A * C], f32, tag="a_g")
                awc8 = pool.tile([128, NI * KO], f32, tag="awc8")
                a8_g = pool.tile([128, NG_A * C], f32, tag="a8_g")
                for g, cnt in ((0, 3), (1, 3), (2, 2)):
                    for ps_t, aw_t, ag_t, w_t in (
                            (p_ps, awc, a_g, w_m), (p8_ps, awc8, a8_g, w_8f)):
                        eng(PROD_ENG).tensor_tensor(
                            out=aw_t[:, g * 3 * KO:(g * 3 + cnt) * KO]
                                .rearrange("p (q c) -> p q c", q=cnt),
                            in0=ps_t[:, g * 512:g * 512 + cnt * KO]
                                .rearrange("p (q c) -> p q c", q=cnt),
                            in1=w_t[:, g * 3 * KO:(g * 3 + cnt) * KO]
                                .rearrange("p (q c) -> p q c", q=cnt),
                            op=ALU.mult)
                        eng(RED_ENG).tensor_reduce(
                            out=ag_t[:, g * C:(g + 1) * C],
                            in_=aw_t[:, g * 3 * KO:(g * 3 + cnt) * KO]
                                .rearrange("p (q k o) -> p k q o", q=cnt, k=C),
                            op=ALU.add, axis=AX.XY)
                a_m = pool.tile([128, C], f32, tag="a_m")
                nc.vector.tensor_add(a_m[:, :], a_g[:, 0:C], a_g[:, C:2 * C])
                nc.vector.tensor_add(a_m[:, :], a_m[:, :],
                                     a_g[:, 2 * C:3 * C])
                a8_red = pool.tile([128, C], f32, tag="a8_red")
                nc.vector.tensor_add(a8_red[:, :], a8_g[:, 0:C],
                                     a8_g[:, C:2 * C])
                nc.vector.tensor_add(a8_red[:, :], a8_red[:, :],
                                     a8_g[:, 2 * C:3 * C])

                def bupd(bt, at):
                    if it == 0:
                        nc.vector.tensor_scalar_mul(bt[:, :], at[:, :],
                                                    1.0 / B)
                    else:
                        nc.vector.tensor_scalar_mul(at[:, :], at[:, :],
                                                    1.0 / B)
                        nc.vector.tensor_add(bt[:, :], bt[:, :], at[:, :])

                def softmax(bt, tagp):
                    mx = pool.tile([128, 1], f32, tag=f"{tagp}mx", bufs=2)
                    e_sb = pool.tile([128, C], f32, tag=f"{tagp}e", bufs=2)
                    sm = pool.tile([128, 1], f32, tag=f"{tagp}s", bufs=2)
                    c_sb = pool.tile([128, C], f32, tag=f"{tagp}c", bufs=2)
                    nc.vector.reduce_max(mx, bt[:, :], axis=AX.X)
                    nc.gpsimd.tensor_sub(
                        e_sb[:, :], bt[:, :], mx.broadcast_to((128, C)))
                    nc.scalar.activation(e_sb, e_sb, ACT.Exp)
                    nc.vector.reduce_sum(sm, e_sb[:, :], axis=AX.X)
                    nc.vector.reciprocal(sm, sm)
                    nc.gpsimd.tensor_mul(
                        c_sb[:, :], e_sb[:, :], sm.broadcast_to((128, C)))
                    return c_sb

                bupd(b_m, a_m)
                c_m = softmax(b_m, "m")
                cmb = c_m.unsqueeze(1).unsqueeze(3)
                half = NI // 2
                eng(WC_ENG[0]).tensor_tensor(
                    out=wcmv[:, :half], in0=wmv[:, :half],
                    in1=cmb.broadcast_to((128, half, C, O)),
                    op=ALU.mult)
                eng(WC_ENG[1]).tensor_tensor(
                    out=wcmv[:, half:], in0=wmv[:, half:],
                    in1=cmb.broadcast_to((128, NI - half, C, O)),
                    op=ALU.mult)
                bupd(b_8, a8_red)
                c_8 = softmax(b_8, "b8")
                eng(WC8_ENG).tensor_tensor(
                    out=wc8.rearrange("p (k o) -> p k o", k=C),
                    in0=w_8.rearrange("p (k o) -> p k o", k=C),
                    in1=c_8.unsqueeze(2).broadcast_to((128, C, O)),
                    op=ALU.mult)

            for _rep in range(repeat):
                v0 = state.pop("v0", None)
                if v0 is None:
                    v0 = emit_front(0)
                emit_a_phase(0, v0)
                v1 = emit_front(1)
                emit_a_phase(1, v1)

                # it2: chain, then prefetch next rep's it0 front so its
                # AR queues right behind it2's and overlaps the tail
                wm_src, w8_src = wc_m, wc8
                s_ps = ps.tile([128, NH * KO], f32, tag="s_ps", bufs=2)
                s_sb = pool.tile([128, NH * KO], f32, tag="s_sb", bufs=2)
                s_pre = pool.tile([128, NH * KO], f32, tag="s_pre", bufs=2)
                for h in range(NH):
                    hs = slice(h * KO, (h + 1) * KO)
                    for i in range(NI):
                        nc.tensor.matmul(
                            s_ps[:, hs],
                            x_m[:, (i * NH + h) * 128:(i * NH + h + 1) * 128],
                            wm_src[:, i * KO:(i + 1) * KO],
                            start=(i == 0), stop=False)
                    nc.tensor.matmul(
                        s_ps[:, hs], x8s[:, h * 128:(h + 1) * 128],
                        w8_src[:, :], start=False, stop=True)
                    nc.scalar.activation(s_pre[:, hs], s_ps[:, hs], ACT.Copy)
                s_ci = dram.tile([128, NH * KO], f32, tag="s_ci")
                nc.sync.dma_start(s_ci[:, :], s_pre[:, :])
                if collective:
                    s_co = dram.tile([128, NH * KO], f32, tag="s_co",
                                     addr_space="Shared")
                    nc.gpsimd.collective_compute(
                        "AllReduce", ALU.add,
                        replica_groups=[list(range(num_devices))],
                        ins=[s_ci.opt()], outs=[s_co.opt()])
                    nc.sync.dma_start(s_sb[:, :], s_co[:, :])
                else:
                    nc.sync.dma_start(s_sb[:, :], s_ci[:, :])
                if _rep + 1 < repeat:
                    state["v0"] = emit_front(0)
                v_sb = pool.tile([128, NH * KO], f32, tag="v_sb", bufs=2)
                nc.scalar.activation(sq_t[:, :], s_sb[:, :], ACT.Square)
                nc.scalar.activation(sg_t[:, :], s_sb[:, :], ACT.Sign)
                nc.vector.tensor_scalar_add(rc_t[:, :], sq_t[:, :], 1.0)
                nc.vector.reciprocal_approx_fast(rc_t[:, :], rc_t[:, :])
                nc.vector.tensor_mul(m_t[:, :], sq_t[:, :], rc_t[:, :])
                nc.vector.tensor_mul(v_sb[:, :], m_t[:, :], sg_t[:, :])
                nc.sync.dma_start(v_d[:, :], v_sb[:, :])

    nc.compile()
    return nc


def _host_prep(x, W):
    x_np = np.asarray(x, np.float32)          # [B, N, I]
    W0 = np.asarray(W[0], np.float32)         # [N, C, O, I]

    # block-8 pieces shared by every core (bar the per-core i-slice)
    x8 = x_np[:, 8 * 128:, :]                 # [B, 128, I]

    in_maps = []
    for c in range(NCORES):
        xn = x_np[:, c * 128:(c + 1) * 128, :]     # [B, 128, I]
        # x_m[p, i, h, bcol] = xn[h*128+bcol, p, i]
        x_m = np.ascontiguousarray(
            xn.reshape(NH, 128, 128, I).transpose(2, 3, 0, 1)
            .reshape(128, I * NH * 128))
        # x_pm[p, i, h, ncol] = xn[h*128+p, ncol, i]
        x_pm = np.ascontiguousarray(
            xn.reshape(NH, 128, 128, I).transpose(1, 3, 0, 2)
            .reshape(128, I * NH * 128))
        # x8s[p, h, bcol] = x8[h*128+bcol, p, c]
        x8s = np.ascontiguousarray(
            x8[:, :, c].reshape(NH, 128, 128).transpose(2, 0, 1)
            .reshape(128, NH * 128))
        # x8p[p, i, h, ncol] = x8[h*128+p, ncol, i]
        x8p = np.ascontiguousarray(
            x8.reshape(NH, 128, 128, I).transpose(1, 3, 0, 2)
            .reshape(128, I * NH * 128))
        # w_m[p, i, k, o] = W0[c*128+p, k, o, i]
        w_m = np.ascontiguousarray(
            W0[c * 128:(c + 1) * 128].transpose(0, 3, 1, 2)
            .reshape(128, I * KO))
        # w_8[p, k, o] = W0[1024+p, k, o, c]; w_8f: all i
        w_8 = np.ascontiguousarray(
            W0[8 * 128:, :, :, c].reshape(128, KO))
        w_8f = np.ascontiguousarray(
            W0[8 * 128:].transpose(0, 3, 1, 2).reshape(128, I * KO))
        in_maps.append({
            "x_m": x_m, "x_pm": x_pm, "x8s": x8s, "x8p": x8p,
            "w_m": w_m, "w_8": w_8, "w_8f": w_8f,
            "w0_m": np.ascontiguousarray(0.1 * w_m),
            "w0_8": np.ascontiguousarray(0.1 * w_8),
        })
    return in_maps


def kernel(x, W):
    from concourse import bass_utils

    if "nc" not in _BUILT:
        _BUILT["nc"] = _build_program()
    nc = _BUILT["nc"]

    in_maps = _host_prep(x, W)
    res = bass_utils.run_bass_kernel_spmd(
        nc, in_maps, core_ids=list(range(NCORES)))
    v = res.results[0]["v_out"]               # [128, 2*160], b = h*128+p
    out = v.reshape(128, NH, KO).transpose(1, 0, 2).reshape(B, C, O, 1)
    return np.ascontiguousarray(out).astype(np.float32)


if __name__ == "__main__":
    rng = np.random.default_rng(0)
    x = rng.standard_normal((B, N, I), np.float32)
    W = rng.standard_normal((1, N, C, O, I), np.float32)
    out = kernel(x, W)
    print(out.shape, out.dtype, np.abs(out).max())
